# revision 20
# baseline (speedup 1.0000x reference)
"""Multi-head causal attention (B=2, S=2048, D=1024, H=16, Dh=64) on 8 TRN2 cores.

Sharding: core = (b, g) with b = batch (2), g = head-group (4 heads each).
Each core computes QKV projections for its batch against its 4 heads' weight
columns, causal attention for those heads, and the partial output projection
against its 4 heads' Wo rows.  Host sums the 4 partials per batch and adds
the bias.

Precision: single-fp8 operands fail the 2e-2 gate everywhere except the
score path -- per-element quantization noise (~2.7% for e4m3) passes through
dot products against random data undamped, and independent contributions
stack to ~5.5e-2.  So:
  scores     Q^T/K^T stored fp8 (the one affordable fp8 noise source,
             ~1.3e-2); DoubleRow with a stride-0 broadcast pair contracts
             dh=64 twice at 0.5 cycles/row (2x bf16; doubling folds into
             the exp scale)
  projections fp8 hi/lo pairs: x and 32*W are each split h + l/16 with both
             halves e4m3; DR pairs (Wh,Wl/16)x(xh,xh) and k-tile-paired
             (Wh)x(xl/16) give ~8-bit effective mantissa at 0.75x the bf16
             cycle count
  AV/out-proj bf16 (at and ctx cannot be hi/lo-split cheaply)

Layouts avoid all on-chip transposes:
  x^T [128, 8k, S] k-tile-major feeds projections directly
  V is projected in [s, dh] orientation (x^T tiles as lhsT), landing
  AV-ready with an appended ones column (row 64 accumulates softmax sums)
  scores are computed transposed [k, q] so exp output feeds AV directly

Engine split: PE does matmuls only; ACT does exp only; DVE handles
PSUM-sourced copies/reciprocals and the normalization multiply; the
otherwise-idle GPSIMD does the causal staircase mask multiplies and the
1/sums partition broadcast (replacing the baseline's rank-1 PE matmuls).
"""

import numpy as np
import ml_dtypes

B = 2
S = 2048
D = 1024
HPC = 4  # heads per core
DH = 64
QB = 512  # q band width
NB = S // QB  # 4 bands
KT = 128  # k tile
N_CORES = 8

SW = 32.0  # host scale on Wq/Wk/Wv (fp8 hi/lo range); V copies undo it
# exp(s_psum * EXP_SCALE) = exp(s_true / sqrt(DH)); the stride-0 DR pair
# doubles s_psum and q/k each carry SW.
EXP_SCALE = 1.0 / (8.0 * 2.0 * SW * SW)

_CACHE = {}


def _build_bass():
    import concourse.bacc as bacc
    import concourse.tile as tile
    from concourse import mybir

    f32 = mybir.dt.float32
    bf16 = mybir.dt.bfloat16
    fp8 = mybir.dt.float8e4
    DR = mybir.MatmulPerfMode.DoubleRow
    ExpF = mybir.ActivationFunctionType.Exp

    nc = bacc.Bacc("TRN2", target_bir_lowering=False)

    xT_d = nc.dram_tensor("xT", [128, 8, 2, S], fp8, kind="ExternalInput")
    wqkv_d = nc.dram_tensor(
        "wqkv", [128, 8, 2, 768], fp8, kind="ExternalInput"
    )
    wo_d = nc.dram_tensor("wo", [128, 2, D], bf16, kind="ExternalInput")
    masks_d = nc.dram_tensor("masks", [128, 4, QB], bf16, kind="ExternalInput")
    out_d = nc.dram_tensor("out", [S, D], bf16, kind="ExternalOutput")

    with tile.TileContext(nc) as tc:
        with (
            tc.tile_pool(name="consts", bufs=1) as consts,
            tc.tile_pool(name="persist", bufs=1) as persist,
            tc.tile_pool(name="score_ps", bufs=2, space="PSUM") as score_ps,
            tc.tile_pool(name="ctx_ps", bufs=2, space="PSUM") as ctx_ps,
            tc.tile_pool(name="misc_ps", bufs=2, space="PSUM") as misc_ps,
            tc.tile_pool(name="at_pool", bufs=8) as at_pool,
            tc.tile_pool(name="rr_pool", bufs=4) as rr_pool,
            tc.tile_pool(name="rb_pool", bufs=4) as rb_pool,
            tc.tile_pool(name="osb_pool", bufs=6) as osb_pool,
        ):
            # ---- constants: weights first (first proj group needs them),
            #      band-0 x^T slices, masks; later x^T bands stream behind ----
            wqkv = consts.tile([128, 8, 2, 768], fp8, tag="wqkv", name="wqkv")
            xT = consts.tile([128, 8, 2, S], fp8, tag="xT", name="xT")
            nc.sync.dma_start(out=wqkv[:, 0, :, :], in_=wqkv_d[:, 0, :, :])
            nc.sync.dma_start(
                out=xT[:, 0:4, :, 0:QB], in_=xT_d[:, 0:4, :, 0:QB]
            )
            nc.sync.dma_start(out=wqkv[:, 1:8, :, :], in_=wqkv_d[:, 1:8, :, :])
            nc.sync.dma_start(
                out=xT[:, 4:8, :, 0:QB], in_=xT_d[:, 4:8, :, 0:QB]
            )
            for j in range(1, NB):
                nc.sync.dma_start(
                    out=xT[:, :, :, j * QB : (j + 1) * QB],
                    in_=xT_d[:, :, :, j * QB : (j + 1) * QB],
                )
            mask_sb = consts.tile([128, 4, QB], bf16, tag="masks", name="masks")
            nc.sync.dma_start(out=mask_sb, in_=masks_d[:, :, :])
            wo = consts.tile([128, 2, D], bf16, tag="wo", name="wo")
            nc.sync.dma_start(out=wo, in_=wo_d[:, :, :])

            # ---- persistent activations ----
            qT = [
                persist.tile([128, S], fp8, tag=f"qT{p}", name=f"qT{p}")
                for p in range(2)
            ]
            kTt = [
                persist.tile([128, S], fp8, tag=f"kT{p}", name=f"kT{p}")
                for p in range(2)
            ]
            # v: (k-position, k-tile, head-in-pair, dh + ones column)
            vp = [
                persist.tile([128, 16, 2, 65], bf16, tag=f"vp{p}", name=f"vp{p}")
                for p in range(2)
            ]
            # ctx^T, normalized: (dh-in-pair, pair, q)
            ctxo = persist.tile([128, 2, S], bf16, tag="ctxo", name="ctxo")
            for p in range(2):
                nc.gpsimd.memset(vp[p][:, :, :, 64:65], 1.0)

            def dr2(ap, n, part=64):
                """View a [part, n] slice as a stride-0 [part, 2, n] pair."""
                return ap.unsqueeze(1).broadcast_to([part, 2, n])

            def emit_qk_chain(j, p, t):
                dest = kTt if t == 1 else qT
                q0 = j * QB
                if True:
                    if True:
                        c0 = 256 * t + 128 * p
                        ps = misc_ps.tile([128, QB], f32, tag="misc", name="pqk")
                        for k in range(8):
                            nc.tensor.matmul(
                                ps,
                                lhsT=wqkv[:, k, :, c0 : c0 + 128],
                                rhs=dr2(xT[:, k, 0, q0 : q0 + QB], QB, 128),
                                start=(k == 0),
                                stop=False,
                                perf_mode=DR,
                            )
                        for t2 in range(4):
                            nc.tensor.matmul(
                                ps,
                                lhsT=wqkv[:, 2 * t2 : 2 * t2 + 2, 0, c0 : c0 + 128],
                                rhs=xT[:, 2 * t2 : 2 * t2 + 2, 1, q0 : q0 + QB],
                                start=False,
                                stop=(t2 == 3),
                                perf_mode=DR,
                            )
                        nc.vector.tensor_copy(
                            out=dest[p][:, q0 : q0 + QB], in_=ps
                        )

            def emit_projQK(j):
                """Q^T/K^T projections for band j: fp8 hi/lo DoubleRow
                chains landing as fp8 [128, QB] slabs (pair rows = 2 heads
                x 64 dh) that feed the DR score matmuls."""
                for p in range(2):
                    for t in (1, 0):
                        emit_qk_chain(j, p, t)

            def emit_projV(j):
                """V for band j's k-tiles, projected directly in [s, dh]
                orientation (x^T tiles as lhsT) -- lands AV-ready, no
                transposes."""
                for kt4 in range(4):
                    kt = 4 * j + kt4
                    for p in range(2):
                        c0 = 512 + 128 * p
                        ps = misc_ps.tile(
                            [128, 2, 64], f32, tag="misc", name="pv"
                        )
                        for k in range(8):
                            nc.tensor.matmul(
                                ps,
                                lhsT=xT[:, k, :, kt * KT : (kt + 1) * KT],
                                rhs=dr2(wqkv[:, k, 0, c0 : c0 + 128], 128, 128),
                                start=(k == 0),
                                stop=False,
                                perf_mode=DR,
                            )
                        for t2 in range(4):
                            nc.tensor.matmul(
                                ps,
                                lhsT=xT[:, 2 * t2 : 2 * t2 + 2, 0, kt * KT : (kt + 1) * KT],
                                rhs=wqkv[:, 2 * t2 : 2 * t2 + 2, 1, c0 : c0 + 128],
                                start=False,
                                stop=(t2 == 3),
                                perf_mode=DR,
                            )
                        nc.vector.tensor_scalar_mul(
                            vp[p][:, kt, :, 0:64], ps, 1.0 / SW
                        )

            def attn_half(j, p, hooks=None):
                """Scores+softmax+AV for band j, head-pair p.

                Scores land transposed ([k, q]) in a [128, 2, QB] fp32 PSUM
                tile per (k-tile-pair, head); one exp covers both halves.
                Diagonal pairs extend the odd k-tile's q-range down to the
                even tile's start so the exp stays a single strided
                instruction; the AV matmuls read per-tile causal ranges so
                the extension region is never consumed.  GPSIMD applies the
                128-wide staircase mask strips after exp."""
                q0 = j * QB
                n_i2 = 2 * (j + 1)
                cps = [
                    ctx_ps.tile([65, QB], f32, tag="ctx", name="ctx")
                    for _ in range(2)
                ]
                for i2 in range(n_i2):
                    o_e = 2 * i2 - 4 * j
                    diag = o_e >= 0
                    z_e = 128 * o_e if diag else 0
                    z_o = z_e + 128 if diag else 0
                    sps_c, at_c = [], []
                    for c in range(2):
                        sps = score_ps.tile(
                            [128, 2, QB], f32, tag="sps", name="sps"
                        )
                        for half in range(2):
                            i = 2 * i2 + half
                            nc.tensor.matmul(
                                sps[:, half, z_e:QB],
                                lhsT=dr2(
                                    kTt[p][
                                        64 * c : 64 * c + 64,
                                        i * KT : (i + 1) * KT,
                                    ],
                                    KT,
                                ),
                                rhs=dr2(
                                    qT[p][
                                        64 * c : 64 * c + 64,
                                        q0 + z_e : q0 + QB,
                                    ],
                                    QB - z_e,
                                ),
                                start=True,
                                stop=True,
                                perf_mode=DR,
                            )
                        sps_c.append(sps)
                    for c in range(2):
                        at = at_pool.tile(
                            [128, 2, QB], bf16, tag="at", name="at"
                        )
                        nc.scalar.activation(
                            out=at[:, :, z_e:QB],
                            in_=sps_c[c][:, :, z_e:QB],
                            func=ExpF,
                            scale=EXP_SCALE,
                        )
                        if diag:
                            nc.gpsimd.tensor_mul(
                                at[:, 0, z_e:z_o],
                                at[:, 0, z_e:z_o],
                                mask_sb[:, o_e, z_e:z_o],
                            )
                            nc.gpsimd.tensor_mul(
                                at[:, 1, z_o : z_o + 128],
                                at[:, 1, z_o : z_o + 128],
                                mask_sb[:, o_e + 1, z_o : z_o + 128],
                            )
                        at_c.append(at)
                    for c in range(2):
                        for half in range(2):
                            i = 2 * i2 + half
                            o = i - 4 * j
                            z = 128 * o if o > 0 else 0
                            nc.tensor.matmul(
                                cps[c][:, z:QB],
                                lhsT=vp[p][:, i, c, :],
                                rhs=at_c[c][:, half, z:QB],
                                start=(i == 0),
                                stop=(i == 4 * (j + 1) - 1),
                            )
                    if hooks:
                        for cl in hooks.get(i2, ()):
                            cl()
                return cps

            def norm_front(j, p, cps):
                """Dependency-free part of ctx normalization: DVE reciprocal
                of the sums row (ctx PSUM row 64) and a ctx copy to SBUF
                (releasing the ctx PSUM bank), plus the GPSIMD partition
                broadcast of 1/sums.  Returns closures for the final
                multiplies, emitted later so they sit behind the projection
                copies in the DVE queue."""
                q0 = j * QB
                mults = []
                for c in range(2):
                    rr = rr_pool.tile([1, QB], bf16, tag="rr", name="rr")
                    with nc.allow_low_precision(
                        reason="reciprocal feeds a bf16 multiply"
                    ):
                        nc.vector.reciprocal(out=rr, in_=cps[c][64:65, :])
                    cf = rb_pool.tile([64, QB], bf16, tag="cf", name="cf")
                    nc.vector.tensor_copy(out=cf, in_=cps[c][0:64, :])
                    rbs = rb_pool.tile([64, QB], bf16, tag="rb", name="rb")
                    nc.gpsimd.partition_broadcast(rbs, rr)
                    mults.append(
                        lambda c=c, cf=cf, rbs=rbs: nc.vector.tensor_mul(
                            ctxo[64 * c : 64 * c + 64, p, q0 : q0 + QB],
                            cf,
                            rbs,
                        )
                    )
                return mults

            def emit_outproj(j):
                last = j == NB - 1
                for m in range(4 * j, 4 * j + 4):
                    osb = osb_pool.tile([128, D], bf16, tag="osb", name="osb")
                    for n in range(2):
                        ops = misc_ps.tile([128, QB], f32, tag="misc", name="ops")
                        for p in range(2):
                            nc.tensor.matmul(
                                ops,
                                lhsT=ctxo[:, p, m * KT : (m + 1) * KT],
                                rhs=wo[:, p, n * QB : (n + 1) * QB],
                                start=(p == 0),
                                stop=(p == 1),
                            )
                        nc.vector.tensor_copy(
                            out=osb[:, n * QB : (n + 1) * QB], in_=ops
                        )
                        if last:
                            nc.sync.dma_start(
                                out=out_d[
                                    m * KT : (m + 1) * KT,
                                    n * QB : (n + 1) * QB,
                                ],
                                in_=osb[:, n * QB : (n + 1) * QB],
                            )
                    if not last:
                        nc.sync.dma_start(
                            out=out_d[m * KT : (m + 1) * KT, :], in_=osb
                        )

            # Coarse software pipeline, one split point per half-band: the
            # next band's Q/K projections sit between the two attention
            # halves and V + out-proj after, so the ACT engine's exp queue
            # never drains while the PE works through its dense blocks.
            emit_projQK(0)
            emit_projV(0)
            prev_op = None
            for j in range(NB):
                hooks = {}
                if j >= 1:
                    hooks[1] = [lambda j=j: emit_projV(j)]
                    if j + 1 < NB:
                        n_i2 = 2 * (j + 1)
                        chains = [
                            (lambda j=j, p=p, t=t: emit_qk_chain(j + 1, p, t))
                            for p in range(2)
                            for t in (1, 0)
                        ]
                        for idx, cl in enumerate(chains):
                            hooks.setdefault(n_i2 - 4 + idx, []).append(cl)
                cps0 = attn_half(j, 0, hooks)
                mults = norm_front(j, 0, cps0)
                if j == 0:
                    emit_projQK(1)
                elif j + 1 >= NB and prev_op is not None:
                    emit_outproj(prev_op)  # band 3 mid: band 2's out-proj
                    prev_op = None
                cps1 = attn_half(j, 1)
                mults += norm_front(j, 1, cps1)
                for mu in mults:
                    mu()
                # out-proj runs one band late so the final, most ACT-bound
                # band still has independent PE work to chew on
                if prev_op is not None:
                    emit_outproj(prev_op)
                prev_op = j
            emit_outproj(NB - 1)

    nc.compile()
    return nc


def _get_bass():
    if "nc" not in _CACHE:
        _CACHE["nc"] = _build_bass()
    return _CACHE["nc"]


def _make_in_maps(x, Wq, Wk, Wv, Wo):
    bf = ml_dtypes.bfloat16
    if "masks" not in _CACHE:
        # causal staircase masks: keep iff q >= k + 128*o  (within a band, a
        # k-tile at offset o*128 above the band start)
        kp = np.arange(128)[:, None]
        qf = np.arange(QB)[None, :]
        _CACHE["masks"] = np.ascontiguousarray(
            np.stack(
                [(qf >= kp + 128 * o).astype(np.float32) for o in range(4)]
            ).transpose(1, 0, 2)
        ).astype(bf)
    masks = _CACHE["masks"]

    e4 = ml_dtypes.float8_e4m3

    def hilo(a, rows):
        """Split f32 [rows, cols] into fp8 (high, low/16) halves, laid out
        [128, rows/128, 2, cols] k-tile-major."""
        ah = a.astype(e4)
        al = ((a - ah.astype(np.float32)) * 16.0).astype(e4)
        al16 = (al.astype(np.float32) / 16.0).astype(e4)
        st = np.stack([ah, al16], axis=1)  # [rows, 2, cols]
        return np.ascontiguousarray(
            st.reshape(rows // 128, 128, 2, -1).transpose(1, 0, 2, 3)
        )

    # x^T in k-tile-major hi/lo layout: (p, k, h, s) from x[b][s, 128k + p]
    xTs = [hilo(x[b].T, D) for b in range(B)]
    in_maps = []
    for core in range(N_CORES):
        b, g = divmod(core, 4)
        hs = slice(g * 256, (g + 1) * 256)
        if core < 4:
            wqkv_f = SW * np.concatenate(
                [Wq[:, hs], Wk[:, hs], Wv[:, hs]], axis=1
            )
            shards = {
                "wqkv": hilo(wqkv_f, D),
                "wo": np.ascontiguousarray(
                    Wo[hs, :].reshape(2, 128, D).transpose(1, 0, 2)
                ).astype(bf),
            }
        else:
            shards = {k: in_maps[core - 4][k] for k in ("wqkv", "wo")}
        in_maps.append({"xT": xTs[b], "masks": masks, **shards})
    return in_maps


def _run(x, Wq, Wk, Wv, Wo, bo, trace=False):
    from concourse.bass_utils import run_bass_kernel_spmd

    nc = _get_bass()
    in_maps = _make_in_maps(x, Wq, Wk, Wv, Wo)
    res = run_bass_kernel_spmd(
        nc, in_maps, core_ids=list(range(N_CORES)), trace=trace
    )
    out = np.zeros((B, S, D), np.float32)
    for core in range(N_CORES):
        out[core // 4] += res.results[core]["out"].astype(np.float32)
    out += bo.astype(np.float32)
    return out, res


def kernel(x, Wq, Wk, Wv, Wo, bo):
    x, Wq, Wk, Wv, Wo, bo = (np.asarray(a) for a in (x, Wq, Wk, Wv, Wo, bo))
    out, _ = _run(x, Wq, Wk, Wv, Wo, bo, trace=False)
    return out


def kernel_traced(x, Wq, Wk, Wv, Wo, bo):
    """Same as kernel() but returns (out, BassKernelResults) with profiling."""
    x, Wq, Wk, Wv, Wo, bo = (np.asarray(a) for a in (x, Wq, Wk, Wv, Wo, bo))
    return _run(x, Wq, Wk, Wv, Wo, bo, trace=True)


# revision 21
# speedup vs baseline: 1.0076x; 1.0076x over previous
"""Multi-head causal attention (B=2, S=2048, D=1024, H=16, Dh=64) on 8 TRN2 cores.

Sharding: core = (b, g) with b = batch (2), g = head-group (4 heads each).
Each core computes QKV projections for its batch against its 4 heads' weight
columns, causal attention for those heads, and the partial output projection
against its 4 heads' Wo rows.  Host sums the 4 partials per batch and adds
the bias.

Precision: single-fp8 operands fail the 2e-2 gate everywhere except the
score path -- per-element quantization noise (~2.7% for e4m3) passes through
dot products against random data undamped, and independent contributions
stack to ~5.5e-2.  So:
  scores     Q^T/K^T stored fp8 (the one affordable fp8 noise source,
             ~1.3e-2); DoubleRow with a stride-0 broadcast pair contracts
             dh=64 twice at 0.5 cycles/row (2x bf16; doubling folds into
             the exp scale)
  projections fp8 hi/lo pairs: x and 32*W are each split h + l/16 with both
             halves e4m3; DR pairs (Wh,Wl/16)x(xh,xh) and k-tile-paired
             (Wh)x(xl/16) give ~8-bit effective mantissa at 0.75x the bf16
             cycle count
  AV/out-proj bf16 (at and ctx cannot be hi/lo-split cheaply)

Layouts avoid all on-chip transposes:
  x^T [128, 8k, S] k-tile-major feeds projections directly
  V is projected in [s, dh] orientation (x^T tiles as lhsT), landing
  AV-ready with an appended ones column (row 64 accumulates softmax sums)
  scores are computed transposed [k, q] so exp output feeds AV directly

Engine split: PE does matmuls only; ACT does exp only; DVE handles
PSUM-sourced copies/reciprocals and the normalization multiply; the
otherwise-idle GPSIMD does the causal staircase mask multiplies and the
1/sums partition broadcast (replacing the baseline's rank-1 PE matmuls).
"""

import numpy as np
import ml_dtypes

B = 2
S = 2048
D = 1024
HPC = 4  # heads per core
DH = 64
QB = 512  # q band width
NB = S // QB  # 4 bands
KT = 128  # k tile
N_CORES = 8

SW = 32.0  # host scale on Wq/Wk/Wv (fp8 hi/lo range); V copies undo it
# exp(s_psum * EXP_SCALE) = exp(s_true / sqrt(DH)); the stride-0 DR pair
# doubles s_psum and q/k each carry SW.
EXP_SCALE = 1.0 / (8.0 * 2.0 * SW * SW)

_CACHE = {}


def _build_bass():
    import concourse.bacc as bacc
    import concourse.tile as tile
    from concourse import mybir

    f32 = mybir.dt.float32
    bf16 = mybir.dt.bfloat16
    fp8 = mybir.dt.float8e4
    DR = mybir.MatmulPerfMode.DoubleRow
    ExpF = mybir.ActivationFunctionType.Exp

    nc = bacc.Bacc("TRN2", target_bir_lowering=False)

    xT_d = nc.dram_tensor("xT", [128, 8, 2, S], fp8, kind="ExternalInput")
    wqkv_d = nc.dram_tensor(
        "wqkv", [128, 8, 2, 768], fp8, kind="ExternalInput"
    )
    wo_d = nc.dram_tensor("wo", [128, 2, D], bf16, kind="ExternalInput")
    masks_d = nc.dram_tensor("masks", [128, 4, QB], bf16, kind="ExternalInput")
    out_d = nc.dram_tensor("out", [S, D], bf16, kind="ExternalOutput")

    with tile.TileContext(nc) as tc:
        with (
            tc.tile_pool(name="consts", bufs=1) as consts,
            tc.tile_pool(name="persist", bufs=1) as persist,
            tc.tile_pool(name="score_ps", bufs=2, space="PSUM") as score_ps,
            tc.tile_pool(name="ctx_ps", bufs=2, space="PSUM") as ctx_ps,
            tc.tile_pool(name="misc_ps", bufs=2, space="PSUM") as misc_ps,
            tc.tile_pool(name="at_pool", bufs=8) as at_pool,
            tc.tile_pool(name="rr_pool", bufs=4) as rr_pool,
            tc.tile_pool(name="rb_pool", bufs=4) as rb_pool,
            tc.tile_pool(name="osb_pool", bufs=6) as osb_pool,
        ):
            # ---- constants: weights first (first proj group needs them),
            #      band-0 x^T slices, masks; later x^T bands stream behind ----
            wqkv = consts.tile([128, 8, 2, 768], fp8, tag="wqkv", name="wqkv")
            xT = consts.tile([128, 8, 2, S], fp8, tag="xT", name="xT")
            nc.sync.dma_start(out=wqkv[:, 0, :, :], in_=wqkv_d[:, 0, :, :])
            nc.sync.dma_start(
                out=xT[:, 0:4, :, 0:QB], in_=xT_d[:, 0:4, :, 0:QB]
            )
            for k in range(1, 4):
                nc.sync.dma_start(out=wqkv[:, k, :, :], in_=wqkv_d[:, k, :, :])
            nc.sync.dma_start(
                out=xT[:, 4:8, :, 0:QB], in_=xT_d[:, 4:8, :, 0:QB]
            )
            for k in range(4, 8):
                nc.sync.dma_start(out=wqkv[:, k, :, :], in_=wqkv_d[:, k, :, :])
            for j in range(1, NB):
                nc.sync.dma_start(
                    out=xT[:, :, :, j * QB : (j + 1) * QB],
                    in_=xT_d[:, :, :, j * QB : (j + 1) * QB],
                )
            mask_sb = consts.tile([128, 4, QB], bf16, tag="masks", name="masks")
            nc.sync.dma_start(out=mask_sb, in_=masks_d[:, :, :])
            wo = consts.tile([128, 2, D], bf16, tag="wo", name="wo")
            nc.sync.dma_start(out=wo, in_=wo_d[:, :, :])

            # ---- persistent activations ----
            qT = [
                persist.tile([128, S], fp8, tag=f"qT{p}", name=f"qT{p}")
                for p in range(2)
            ]
            kTt = [
                persist.tile([128, S], fp8, tag=f"kT{p}", name=f"kT{p}")
                for p in range(2)
            ]
            # v: (k-position, k-tile, head-in-pair, dh + ones column)
            vp = [
                persist.tile([128, 16, 2, 65], bf16, tag=f"vp{p}", name=f"vp{p}")
                for p in range(2)
            ]
            # ctx^T, normalized: (dh-in-pair, pair, q)
            ctxo = persist.tile([128, 2, S], bf16, tag="ctxo", name="ctxo")
            for p in range(2):
                nc.gpsimd.memset(vp[p][:, :, :, 64:65], 1.0)

            def dr2(ap, n, part=64):
                """View a [part, n] slice as a stride-0 [part, 2, n] pair."""
                return ap.unsqueeze(1).broadcast_to([part, 2, n])

            def emit_projQK(j):
                """Q^T/K^T projections for band j: bf16 matmuls landing as
                fp8 [128, QB] slabs (pair rows = 2 heads x 64 dh) that feed
                the DoubleRow score matmuls."""
                q0 = j * QB
                for p in range(2):
                    for t, dest in ((1, kTt), (0, qT)):
                        c0 = 256 * t + 128 * p
                        ps = misc_ps.tile([128, QB], f32, tag="misc", name="pqk")
                        for k in range(8):
                            nc.tensor.matmul(
                                ps,
                                lhsT=wqkv[:, k, :, c0 : c0 + 128],
                                rhs=dr2(xT[:, k, 0, q0 : q0 + QB], QB, 128),
                                start=(k == 0),
                                stop=False,
                                perf_mode=DR,
                            )
                        for t2 in range(4):
                            nc.tensor.matmul(
                                ps,
                                lhsT=wqkv[:, 2 * t2 : 2 * t2 + 2, 0, c0 : c0 + 128],
                                rhs=xT[:, 2 * t2 : 2 * t2 + 2, 1, q0 : q0 + QB],
                                start=False,
                                stop=(t2 == 3),
                                perf_mode=DR,
                            )
                        nc.vector.tensor_copy(
                            out=dest[p][:, q0 : q0 + QB], in_=ps
                        )

            def emit_projV(j):
                """V for band j's k-tiles, projected directly in [s, dh]
                orientation (x^T tiles as lhsT) -- lands AV-ready, no
                transposes."""
                for kt4 in range(4):
                    kt = 4 * j + kt4
                    for p in range(2):
                        c0 = 512 + 128 * p
                        ps = misc_ps.tile(
                            [128, 2, 64], f32, tag="misc", name="pv"
                        )
                        for k in range(8):
                            nc.tensor.matmul(
                                ps,
                                lhsT=xT[:, k, :, kt * KT : (kt + 1) * KT],
                                rhs=dr2(wqkv[:, k, 0, c0 : c0 + 128], 128, 128),
                                start=(k == 0),
                                stop=False,
                                perf_mode=DR,
                            )
                        for t2 in range(4):
                            nc.tensor.matmul(
                                ps,
                                lhsT=xT[:, 2 * t2 : 2 * t2 + 2, 0, kt * KT : (kt + 1) * KT],
                                rhs=wqkv[:, 2 * t2 : 2 * t2 + 2, 1, c0 : c0 + 128],
                                start=False,
                                stop=(t2 == 3),
                                perf_mode=DR,
                            )
                        nc.vector.tensor_scalar_mul(
                            vp[p][:, kt, :, 0:64], ps, 1.0 / SW
                        )

            def attn_half(j, p, mid_hook=None):
                """Scores+softmax+AV for band j, head-pair p.

                Scores land transposed ([k, q]) in a [128, 2, QB] fp32 PSUM
                tile per (k-tile-pair, head); one exp covers both halves.
                Diagonal pairs extend the odd k-tile's q-range down to the
                even tile's start so the exp stays a single strided
                instruction; the AV matmuls read per-tile causal ranges so
                the extension region is never consumed.  GPSIMD applies the
                128-wide staircase mask strips after exp."""
                q0 = j * QB
                n_i2 = 2 * (j + 1)
                cps = [
                    ctx_ps.tile([65, QB], f32, tag="ctx", name="ctx")
                    for _ in range(2)
                ]
                for i2 in range(n_i2):
                    o_e = 2 * i2 - 4 * j
                    diag = o_e >= 0
                    z_e = 128 * o_e if diag else 0
                    z_o = z_e + 128 if diag else 0
                    sps_c, at_c = [], []
                    for c in range(2):
                        sps = score_ps.tile(
                            [128, 2, QB], f32, tag="sps", name="sps"
                        )
                        for half in range(2):
                            i = 2 * i2 + half
                            nc.tensor.matmul(
                                sps[:, half, z_e:QB],
                                lhsT=dr2(
                                    kTt[p][
                                        64 * c : 64 * c + 64,
                                        i * KT : (i + 1) * KT,
                                    ],
                                    KT,
                                ),
                                rhs=dr2(
                                    qT[p][
                                        64 * c : 64 * c + 64,
                                        q0 + z_e : q0 + QB,
                                    ],
                                    QB - z_e,
                                ),
                                start=True,
                                stop=True,
                                perf_mode=DR,
                            )
                        sps_c.append(sps)
                    for c in range(2):
                        at = at_pool.tile(
                            [128, 2, QB], bf16, tag="at", name="at"
                        )
                        nc.scalar.activation(
                            out=at[:, :, z_e:QB],
                            in_=sps_c[c][:, :, z_e:QB],
                            func=ExpF,
                            scale=EXP_SCALE,
                        )
                        if diag:
                            nc.gpsimd.tensor_mul(
                                at[:, 0, z_e:z_o],
                                at[:, 0, z_e:z_o],
                                mask_sb[:, o_e, z_e:z_o],
                            )
                            nc.gpsimd.tensor_mul(
                                at[:, 1, z_o : z_o + 128],
                                at[:, 1, z_o : z_o + 128],
                                mask_sb[:, o_e + 1, z_o : z_o + 128],
                            )
                        at_c.append(at)
                    for c in range(2):
                        for half in range(2):
                            i = 2 * i2 + half
                            o = i - 4 * j
                            z = 128 * o if o > 0 else 0
                            nc.tensor.matmul(
                                cps[c][:, z:QB],
                                lhsT=vp[p][:, i, c, :],
                                rhs=at_c[c][:, half, z:QB],
                                start=(i == 0),
                                stop=(i == 4 * (j + 1) - 1),
                            )
                    if i2 == 1 and mid_hook is not None:
                        mid_hook()
                return cps

            def norm_front(j, p, cps):
                """Dependency-free part of ctx normalization: DVE reciprocal
                of the sums row (ctx PSUM row 64) and a ctx copy to SBUF
                (releasing the ctx PSUM bank), plus the GPSIMD partition
                broadcast of 1/sums.  Returns closures for the final
                multiplies, emitted later so they sit behind the projection
                copies in the DVE queue."""
                q0 = j * QB
                mults = []
                for c in range(2):
                    rr = rr_pool.tile([1, QB], bf16, tag="rr", name="rr")
                    with nc.allow_low_precision(
                        reason="reciprocal feeds a bf16 multiply"
                    ):
                        nc.vector.reciprocal(out=rr, in_=cps[c][64:65, :])
                    cf = rb_pool.tile([64, QB], bf16, tag="cf", name="cf")
                    nc.vector.tensor_copy(out=cf, in_=cps[c][0:64, :])
                    rbs = rb_pool.tile([64, QB], bf16, tag="rb", name="rb")
                    nc.gpsimd.partition_broadcast(rbs, rr)
                    mults.append(
                        lambda c=c, cf=cf, rbs=rbs: nc.vector.tensor_mul(
                            ctxo[64 * c : 64 * c + 64, p, q0 : q0 + QB],
                            cf,
                            rbs,
                        )
                    )
                return mults

            def emit_outproj(j):
                last = j == NB - 1
                for m in range(4 * j, 4 * j + 4):
                    osb = osb_pool.tile([128, D], bf16, tag="osb", name="osb")
                    for n in range(2):
                        if last:
                            # scores are done; borrow the idle score-PSUM
                            # pool so the tail's 8 chains don't serialize
                            # through the 2 misc slots
                            ops = score_ps.tile(
                                [128, 2, QB], f32, tag="sps", name="opl"
                            )[:, 0, :]
                        else:
                            ops = misc_ps.tile(
                                [128, QB], f32, tag="misc", name="ops"
                            )
                        for p in range(2):
                            nc.tensor.matmul(
                                ops,
                                lhsT=ctxo[:, p, m * KT : (m + 1) * KT],
                                rhs=wo[:, p, n * QB : (n + 1) * QB],
                                start=(p == 0),
                                stop=(p == 1),
                            )
                        nc.vector.tensor_copy(
                            out=osb[:, n * QB : (n + 1) * QB], in_=ops
                        )
                        if last:
                            nc.sync.dma_start(
                                out=out_d[
                                    m * KT : (m + 1) * KT,
                                    n * QB : (n + 1) * QB,
                                ],
                                in_=osb[:, n * QB : (n + 1) * QB],
                            )
                    if not last:
                        nc.sync.dma_start(
                            out=out_d[m * KT : (m + 1) * KT, :], in_=osb
                        )

            # Coarse software pipeline, one split point per half-band: the
            # next band's Q/K projections sit between the two attention
            # halves and V + out-proj after, so the ACT engine's exp queue
            # never drains while the PE works through its dense blocks.
            emit_projQK(0)
            emit_projV(0)
            prev_op = None
            for j in range(NB):
                hook = (lambda j=j: emit_projV(j)) if j >= 1 else None
                cps0 = attn_half(j, 0, hook)
                mults = norm_front(j, 0, cps0)
                if j + 1 < NB:
                    emit_projQK(j + 1)
                elif prev_op is not None:
                    emit_outproj(prev_op)  # band 3 mid: band 2's out-proj
                    prev_op = None
                cps1 = attn_half(j, 1)
                mults += norm_front(j, 1, cps1)
                for mu in mults:
                    mu()
                # out-proj runs one band late so the final, most ACT-bound
                # band still has independent PE work to chew on
                if prev_op is not None:
                    emit_outproj(prev_op)
                prev_op = j
            emit_outproj(NB - 1)

    nc.compile()
    return nc


def _get_bass():
    if "nc" not in _CACHE:
        _CACHE["nc"] = _build_bass()
    return _CACHE["nc"]


def _make_in_maps(x, Wq, Wk, Wv, Wo):
    bf = ml_dtypes.bfloat16
    if "masks" not in _CACHE:
        # causal staircase masks: keep iff q >= k + 128*o  (within a band, a
        # k-tile at offset o*128 above the band start)
        kp = np.arange(128)[:, None]
        qf = np.arange(QB)[None, :]
        _CACHE["masks"] = np.ascontiguousarray(
            np.stack(
                [(qf >= kp + 128 * o).astype(np.float32) for o in range(4)]
            ).transpose(1, 0, 2)
        ).astype(bf)
    masks = _CACHE["masks"]

    e4 = ml_dtypes.float8_e4m3

    def hilo(a, rows):
        """Split f32 [rows, cols] into fp8 (high, low/16) halves, laid out
        [128, rows/128, 2, cols] k-tile-major."""
        ah = a.astype(e4)
        al = ((a - ah.astype(np.float32)) * 16.0).astype(e4)
        al16 = (al.astype(np.float32) / 16.0).astype(e4)
        st = np.stack([ah, al16], axis=1)  # [rows, 2, cols]
        return np.ascontiguousarray(
            st.reshape(rows // 128, 128, 2, -1).transpose(1, 0, 2, 3)
        )

    # x^T in k-tile-major hi/lo layout: (p, k, h, s) from x[b][s, 128k + p]
    xTs = [hilo(x[b].T, D) for b in range(B)]
    in_maps = []
    for core in range(N_CORES):
        b, g = divmod(core, 4)
        hs = slice(g * 256, (g + 1) * 256)
        if core < 4:
            wqkv_f = SW * np.concatenate(
                [Wq[:, hs], Wk[:, hs], Wv[:, hs]], axis=1
            )
            shards = {
                "wqkv": hilo(wqkv_f, D),
                "wo": np.ascontiguousarray(
                    Wo[hs, :].reshape(2, 128, D).transpose(1, 0, 2)
                ).astype(bf),
            }
        else:
            shards = {k: in_maps[core - 4][k] for k in ("wqkv", "wo")}
        in_maps.append({"xT": xTs[b], "masks": masks, **shards})
    return in_maps


def _run(x, Wq, Wk, Wv, Wo, bo, trace=False):
    from concourse.bass_utils import run_bass_kernel_spmd

    nc = _get_bass()
    in_maps = _make_in_maps(x, Wq, Wk, Wv, Wo)
    res = run_bass_kernel_spmd(
        nc, in_maps, core_ids=list(range(N_CORES)), trace=trace
    )
    out = np.zeros((B, S, D), np.float32)
    for core in range(N_CORES):
        out[core // 4] += res.results[core]["out"].astype(np.float32)
    out += bo.astype(np.float32)
    return out, res


def kernel(x, Wq, Wk, Wv, Wo, bo):
    x, Wq, Wk, Wv, Wo, bo = (np.asarray(a) for a in (x, Wq, Wk, Wv, Wo, bo))
    out, _ = _run(x, Wq, Wk, Wv, Wo, bo, trace=False)
    return out


def kernel_traced(x, Wq, Wk, Wv, Wo, bo):
    """Same as kernel() but returns (out, BassKernelResults) with profiling."""
    x, Wq, Wk, Wv, Wo, bo = (np.asarray(a) for a in (x, Wq, Wk, Wv, Wo, bo))
    return _run(x, Wq, Wk, Wv, Wo, bo, trace=True)
